# revision 5
# baseline (speedup 1.0000x reference)
"""Trainium2 Bass kernel for nn_AdaptiveAttention (dense_cnn, memory-bound).

out[b,c,h,w] = x[b,c,h,w] * (w0*ca[b,c] + w1*sa[b,h,w])

  ca = sigmoid(w2 @ silu(GN(w1 @ mean_hw(x) + b1)) + b2)      (channel attention)
  sa = sigmoid(conv7x7([mean_c(x), max_c(x)]) + sa_b)         (spatial attention)
  (w0, w1) = softmax(balance)

Data-parallel over batch: 8 NeuronCores x 4 batches each.

v4 design (CoreSim cost-model driven rebalance of v3):
  - channel max AND channel mean both via GPSIMD partition_all_reduce
    (1 elem/lane/cycle at 1.2 GHz): m=max(xt0,xt1), s=xt0+xt1 on DVE,
    then PAR(max)/PAR(add) on Pool; row 0 DMA'd into the zero-padded
    conv inputs. This deletes v3's 33 PE transposes, 24 DVE reduce_max,
    the avg-map PE matmuls and all 32 single-partition psum copies.
    The 1/C of the mean folds into the conv's avg-half weights.
  - apply: s_h = ACT(sab, Relu, bias=w0*ca_h) then one DVE bf16 mul per
    half; sab is the [1,4096] sa row broadcast via DRAM bounce re-read
    with a stride-0 partition AP (cheapest 128-way broadcast on TRN2).
  - DMA spread over 4 rings: x loads on sync, sab reads on scalar,
    stores split tensor/sync, pads/scr/consts on gpsimd.
  - squeeze (w1 @ mean_hw) stays on PE: 16 accumulating [16,512] psum
    matmuls per batch; GroupNorm(1)+SiLU chain 4-wide fp32 with a
    DVE-only Quake rsqrt; ca as per-half [128,1] columns.
"""

import sys

import numpy as np

if "/opt/trn_rl_repo" not in sys.path:
    sys.path.insert(0, "/opt/trn_rl_repo")

B, C, H, W = 32, 256, 64, 64
HW = H * W
CR = C // 16
NCORES = 8
BPC = B // NCORES
FREE = BPC * HW
CHUNK = 512
GN_EPS = 1e-5

_BUILT = None


def _build(loop_n=None):
    """Build (once) the SPMD Bass graph for one core's [256, 4*4096] bf16 shard."""
    global _BUILT
    if loop_n is None and _BUILT is not None:
        return _BUILT

    import concourse.bacc as bacc
    import concourse.tile as tile
    from concourse import mybir
    from concourse import bass_isa
    from concourse.bass import AP

    DT = mybir.dt.float32
    BT = mybir.dt.bfloat16
    AF = mybir.ActivationFunctionType
    AL = mybir.AluOpType
    AX = mybir.AxisListType

    nc = bacc.Bacc(
        "TRN2",
        target_bir_lowering=False,
        debug=False,
        enable_asserts=False,
        num_devices=NCORES,
    )

    x_d = nc.dram_tensor("x", [C, FREE], BT, kind="ExternalInput")
    w1sb_d = nc.dram_tensor("w1sb", [128, 2 * CR], BT, kind="ExternalInput")
    b1col_d = nc.dram_tensor("b1col", [CR, 1], DT, kind="ExternalInput")
    gng_d = nc.dram_tensor("gng", [CR, 1], DT, kind="ExternalInput")
    gnb_d = nc.dram_tensor("gnb", [CR, 1], DT, kind="ExternalInput")
    sixt_d = nc.dram_tensor("sixt", [CR, 1], DT, kind="ExternalInput")
    zcol_d = nc.dram_tensor("zcol", [128, 1], DT, kind="ExternalInput")
    w2b_d = nc.dram_tensor("w2b", [CR + 1, C], DT, kind="ExternalInput")
    mconv_d = nc.dram_tensor("mconv", [70, 14 * 64], BT, kind="ExternalInput")
    sabcol_d = nc.dram_tensor("sabcol", [64, 1], DT, kind="ExternalInput")
    w1col_d = nc.dram_tensor("w1col", [64, 1], DT, kind="ExternalInput")
    w0col_d = nc.dram_tensor("w0col", [128, 1], DT, kind="ExternalInput")
    onesr_d = nc.dram_tensor("onesr", [1, C], DT, kind="ExternalInput")
    cent_d = nc.dram_tensor("cent", [CR, CR], DT, kind="ExternalInput")
    scr_d = nc.dram_tensor("scr", [BPC, HW], BT, kind="Internal")
    out_d = nc.dram_tensor("out", [C, FREE], BT, kind="ExternalOutput")

    with tile.TileContext(nc) as tc:
        with (
            tc.tile_pool(name="const", bufs=1) as constp,
            tc.tile_pool(name="xp", bufs=8) as xp,
            tc.tile_pool(name="outp", bufs=3) as outp,
            tc.tile_pool(name="prep", bufs=3) as prep,
            tc.tile_pool(name="parp", bufs=2) as parp,
            tc.tile_pool(name="sabp", bufs=2) as sabp,
            tc.tile_pool(name="sp", bufs=4) as sp,
            tc.tile_pool(name="phps", bufs=2, space="PSUM") as phps,
            tc.tile_pool(name="psml", bufs=2, space="PSUM") as psml,
            tc.tile_pool(name="pconv", bufs=2, space="PSUM") as pconv,
        ):
            # ---- constants (DMA'd once, off the hot rings) ----
            w1sb_t = constp.tile([128, 2 * CR], BT, name="w1sb_t")
            nc.gpsimd.dma_start(out=w1sb_t[:], in_=w1sb_d[:])
            b1col_t = constp.tile([CR, 1], DT, name="b1col_t")
            nc.gpsimd.dma_start(out=b1col_t[:], in_=b1col_d[:])
            gng_t = constp.tile([CR, 1], DT, name="gng_t")
            nc.gpsimd.dma_start(out=gng_t[:], in_=gng_d[:])
            gnb_t = constp.tile([CR, 1], DT, name="gnb_t")
            nc.gpsimd.dma_start(out=gnb_t[:], in_=gnb_d[:])
            sixt_t = constp.tile([CR, 1], DT, name="sixt_t")
            nc.gpsimd.dma_start(out=sixt_t[:], in_=sixt_d[:])
            zcol_t = constp.tile([128, 1], DT, name="zcol_t")
            nc.gpsimd.dma_start(out=zcol_t[:], in_=zcol_d[:])
            w2b_t = constp.tile([CR + 1, C], DT, name="w2b_t")
            nc.gpsimd.dma_start(out=w2b_t[:], in_=w2b_d[:])
            mconv_t = constp.tile([70, 14 * 64], BT, name="mconv_t")
            nc.gpsimd.dma_start(out=mconv_t[:], in_=mconv_d[:])
            sabcol_t = constp.tile([64, 1], DT, name="sabcol_t")
            nc.gpsimd.dma_start(out=sabcol_t[:], in_=sabcol_d[:])
            w1col_t = constp.tile([64, 1], DT, name="w1col_t")
            nc.gpsimd.dma_start(out=w1col_t[:], in_=w1col_d[:])
            w0col_t = constp.tile([128, 1], DT, name="w0col_t")
            nc.gpsimd.dma_start(out=w0col_t[:], in_=w0col_d[:])
            onesr_t = constp.tile([1, C], DT, name="onesr_t")
            nc.gpsimd.dma_start(out=onesr_t[:], in_=onesr_d[:])
            cent16_t = constp.tile([CR, CR], DT, name="cent16_t")
            nc.gpsimd.dma_start(out=cent16_t[:], in_=cent_d[:])

            # persistent zero-bordered conv inputs (interior rewritten per batch)
            pads_av, pads_mx = [], []
            for b in range(BPC):
                pad_av = constp.tile([70, 70], BT, name=f"padav_{b}")
                nc.gpsimd.memset(pad_av[:, :], 0.0)
                pad_mx = constp.tile([70, 70], BT, name=f"padmx_{b}")
                nc.gpsimd.memset(pad_mx[:, :], 0.0)
                pads_av.append(pad_av); pads_mx.append(pad_mx)
            hb4 = constp.tile([CR + 1, BPC], DT, name="hb4")
            nc.gpsimd.dma_start(out=hb4[CR : CR + 1, :], in_=onesr_d[0:1, 0:BPC])

            import contextlib
            loop_cm = tc.For_i(0, loop_n, 1) if loop_n is not None else contextlib.nullcontext()
            with loop_cm:
              hred4 = sp.tile([CR, BPC], DT, tag="hred4", name="hred4", bufs=2)
              xts, sabs = [], []
              for b in range(BPC):
                pad_av, pad_mx = pads_av[b], pads_mx[b]
                bsl = slice(b * HW, (b + 1) * HW)
                xt0 = xp.tile([128, HW], BT, tag="xt", name=f"xt0_{b}")
                xt1 = xp.tile([128, HW], BT, tag="xt", name=f"xt1_{b}")
                nc.sync.dma_start(out=xt0[:, :], in_=x_d[0:128, bsl])
                nc.sync.dma_start(out=xt1[:, :], in_=x_d[128:256, bsl])

                # channel max / channel sum maps via GPSIMD partition reduce
                m = prep.tile([128, HW], BT, tag="pre", name=f"m_{b}")
                nc.vector.tensor_max(m[:, :], xt0[:, :], xt1[:, :])
                mx = parp.tile([128, HW], BT, tag="par", name=f"mx_{b}")
                nc.gpsimd.partition_all_reduce(
                    mx[:, :], m[:, :], channels=128, reduce_op=bass_isa.ReduceOp.max
                )
                nc.gpsimd.dma_start(out=pad_mx[3:67, 3:67], in_=mx[0:1, :])

                s = prep.tile([128, HW], BT, tag="pre", name=f"s_{b}")
                nc.vector.tensor_add(s[:, :], xt0[:, :], xt1[:, :])
                sm = parp.tile([128, HW], BT, tag="par", name=f"sm_{b}")
                nc.gpsimd.partition_all_reduce(
                    sm[:, :], s[:, :], channels=128, reduce_op=bass_isa.ReduceOp.add
                )
                nc.gpsimd.dma_start(out=pad_av[3:67, 3:67], in_=sm[0:1, :])

                # squeeze: h_partial[CR, 512] accumulated over halves+chunks
                hps = phps.tile([CR, CHUNK], DT, tag="hps", name=f"hps_{b}")
                nmm = 0
                for j in range(8):
                    sl = slice(j * CHUNK, (j + 1) * CHUNK)
                    nc.tensor.matmul(
                        hps[:, :], w1sb_t[:, 0:CR], xt0[:, sl],
                        start=(nmm == 0), stop=False,
                    )
                    nmm += 1
                    nc.tensor.matmul(
                        hps[:, :], w1sb_t[:, CR : 2 * CR], xt1[:, sl],
                        start=False, stop=(nmm == 15),
                    )
                    nmm += 1
                nc.vector.reduce_sum(hred4[:, b : b + 1], hps[:, :], axis=AX.X)

                # ---- spatial attention: 7x7 conv as 14 banded bf16 matmuls ----
                convps = pconv.tile([64, 64], DT, tag="conv", name=f"convps_{b}")
                idx = 0
                for ci, pad in ((0, pad_av), (1, pad_mx)):
                    for dw in range(7):
                        jj = ci * 7 + dw
                        nc.tensor.matmul(
                            convps[:, :],
                            mconv_t[:, 64 * jj : 64 * (jj + 1)],
                            pad[:, dw : dw + 64],
                            start=(idx == 0),
                            stop=(idx == 13),
                        )
                        idx += 1
                sasig = sp.tile([64, 64], BT, tag="sasig", name=f"sasig_{b}")
                nc.scalar.activation(sasig[:], convps[:], AF.Sigmoid, bias=sabcol_t[:])
                sasw = sp.tile([64, 64], BT, tag="sasw", name=f"sasw_{b}")
                nc.vector.tensor_scalar_mul(sasw[:], sasig[:], w1col_t[:, 0:1])

                # broadcast w1*sa to all 128 partitions via DRAM bounce
                nc.gpsimd.dma_start(out=scr_d[b : b + 1, :], in_=sasw[:, :])
                sab = sabp.tile([128, HW], BT, tag="sab", name=f"sab_{b}")
                src = scr_d[b : b + 1, :]
                bsrc = AP(src.tensor, src.offset, [[0, 128]] + list(src.ap)[1:])
                nc.scalar.dma_start(out=sab[:, :], in_=bsrc)
                xts.append((xt0, xt1)); sabs.append(sab)

              # ---- 4-wide GroupNorm(1)+SiLU chain (all batches at once) ----
              hcol4 = sp.tile([CR, BPC], DT, tag="hcol4", name="hcol4")
              nc.vector.tensor_scalar_add(hcol4[:, :], hred4[:, :], b1col_t[:, 0:1])
              dps4 = psml.tile([CR, BPC], DT, tag="sps", name="dps4")
              nc.tensor.matmul(dps4[:, :], cent16_t[:, :], hcol4[:, :])
              diff4 = sp.tile([CR, BPC], DT, tag="diff4", name="diff4")
              nc.scalar.copy(diff4[:], dps4[:])
              sq4 = sp.tile([CR, BPC], DT, tag="sq4", name="sq4")
              nc.vector.tensor_mul(sq4[:], diff4[:], diff4[:])
              vrow = psml.tile([1, BPC], DT, tag="sps", name="vrow")
              nc.tensor.matmul(vrow[:, :], sixt_t[:, :], sq4[:, :])
              vsb4 = sp.tile([1, BPC], DT, tag="vsb4", name="vsb4")
              nc.scalar.copy(vsb4[:], vrow[:])
              vm4 = sp.tile([1, BPC], DT, tag="vm4", name="vm4")
              nc.vector.tensor_scalar(vm4[:], vsb4[:], 1.0, GN_EPS, op0=AL.mult, op1=AL.add)
              vh4 = sp.tile([1, BPC], DT, tag="vh4", name="vh4")
              nc.vector.tensor_scalar_mul(vh4[:], vm4[:], 0.5)
              # Quake rsqrt on the [1,4] row; two Newton steps (sign restores)
              rs4 = sp.tile([1, BPC], DT, tag="rs4", name="rs4")
              rs4_i = rs4.bitcast(mybir.dt.int32)
              vm4_i = vm4.bitcast(mybir.dt.int32)
              nc.vector.tensor_scalar(rs4_i[:], vm4_i[:], 1, None, op0=AL.arith_shift_right)
              nc.vector.tensor_scalar(rs4_i[:], rs4_i[:], 0x5F3759DF, -1, op0=AL.subtract, op1=AL.mult)
              for it in range(2):
                  ysq4 = sp.tile([1, BPC], DT, tag="ysq4", name=f"ysq4_{it}")
                  nc.vector.tensor_mul(ysq4[:], rs4[:], rs4[:])
                  u4 = sp.tile([1, BPC], DT, tag="u4", name=f"u4_{it}")
                  nc.vector.tensor_mul(u4[:], ysq4[:], vh4[:])
                  nc.vector.tensor_scalar(u4[:], u4[:], 1.5, None, op0=AL.subtract)
                  nc.vector.tensor_mul(rs4[:], rs4[:], u4[:])
              rs16 = psml.tile([CR, BPC], DT, tag="sps", name="rs16")
              nc.tensor.matmul(rs16[:, :], onesr_t[0:1, 0:CR], rs4[:, :])
              rsc4 = sp.tile([CR, BPC], DT, tag="rsc4", name="rsc4")
              nc.scalar.copy(rsc4[:], rs16[:])
              hn4 = sp.tile([CR, BPC], DT, tag="hn4", name="hn4")
              nc.vector.tensor_mul(hn4[:], diff4[:], rsc4[:])
              hg24 = sp.tile([CR, BPC], DT, tag="hg24", name="hg24")
              nc.vector.tensor_scalar(hg24[:], hn4[:], gng_t[:, 0:1], gnb_t[:, 0:1], op0=AL.mult, op1=AL.add)
              sg4 = sp.tile([CR, BPC], DT, tag="sg4", name="sg4")
              nc.scalar.activation(sg4[:], hg24[:], AF.Sigmoid, bias=zcol_t[0:CR, 0:1])
              nc.vector.tensor_mul(hb4[0:CR, :], hg24[:], sg4[:])

              # ca columns for all batches: [128, 4] per half, kept fp32
              caw4 = []
              for h in range(2):
                  capsh = psml.tile([128, BPC], DT, tag="sps", name=f"caps4_{h}")
                  nc.tensor.matmul(capsh[:, :], w2b_t[:, 128 * h : 128 * (h + 1)], hb4[:, :])
                  casgh = sp.tile([128, BPC], DT, tag=f"casg{h}", name=f"casg4_{h}")
                  nc.scalar.activation(casgh[:], capsh[:], AF.Sigmoid, bias=zcol_t[:, 0:1])
                  cwh = sp.tile([128, BPC], DT, tag=f"caw{h}", name=f"caw4_{h}")
                  nc.vector.tensor_scalar_mul(cwh[:], casgh[:], w0col_t[:, 0:1])
                  caw4.append(cwh)

              # ---- apply: S = relu(sab + w0*ca) on ACT, one bf16 DVE mul per
              # half; stores split across the tensor and sync rings.
              for b in range(BPC):
                  bsl = slice(b * HW, (b + 1) * HW)
                  xt0, xt1 = xts[b]
                  sab = sabs[b]
                  s0 = outp.tile([128, HW], BT, tag="s0", name=f"s0_{b}", bufs=2)
                  nc.scalar.activation(s0[:, :], sab[:, :], AF.Relu, bias=caw4[0][:, b : b + 1])
                  s1 = outp.tile([128, HW], BT, tag="s1", name=f"s1_{b}", bufs=2)
                  nc.vector.tensor_scalar_add(s1[:, :], sab[:, :], caw4[1][:, b : b + 1])
                  ot0 = outp.tile([128, HW], BT, tag="ot", name=f"ot0_{b}")
                  ot1 = outp.tile([128, HW], BT, tag="ot", name=f"ot1_{b}")
                  nc.vector.tensor_mul(ot0[:, :], s0[:, :], xt0[:, :])
                  nc.vector.tensor_mul(ot1[:, :], s1[:, :], xt1[:, :])
                  nc.scalar.dma_start(out=out_d[0:128, bsl], in_=ot0[:, :])
                  nc.sync.dma_start(out=out_d[128:256, bsl], in_=ot1[:, :])

    nc.compile()
    if loop_n is None:
        _BUILT = nc
    return nc


def _host_prep(inputs):
    """Host-side prep of the tiny weight tensors into matmul-ready layouts."""
    import ml_dtypes

    bf16 = ml_dtypes.bfloat16

    w1 = np.asarray(inputs["w1"], np.float32)
    b1 = np.asarray(inputs["b1"], np.float32)
    gn_g = np.asarray(inputs["gn_g"], np.float32)
    gn_b = np.asarray(inputs["gn_b"], np.float32)
    w2 = np.asarray(inputs["w2"], np.float32)
    b2 = np.asarray(inputs["b2"], np.float32)
    sa_w = np.asarray(inputs["sa_w"], np.float32)
    sa_b = np.asarray(inputs["sa_b"], np.float32)
    balance = np.asarray(inputs["balance"], np.float64)

    e = np.exp(balance - balance.max())
    wsm = e / e.sum()
    w0f, w1f = float(wsm[0]), float(wsm[1])

    w1sb = np.zeros((128, 2 * CR), np.float32)
    for ct in range(2):
        w1sb[:, CR * ct : CR * (ct + 1)] = w1[:, 128 * ct : 128 * (ct + 1)].T / HW
    w1sb = w1sb.astype(bf16)
    b1col = b1.reshape(CR, 1).copy()
    gng = gn_g.reshape(CR, 1).copy()
    gnb = gn_b.reshape(CR, 1).copy()
    w2b = np.concatenate([w2.T, b2.reshape(1, C)], axis=0).astype(np.float32)

    # banded H-conv matrices: M_{c,dw}[k, h] = sa_w[0, c, k-h, dw], 0<=k-h<7
    # the avg-channel (c=0) absorbs the 1/C of mean_c (conv input is the SUM map)
    mconv = np.zeros((70, 14, 64), np.float32)
    hh = np.arange(64)
    for c in range(2):
        cscale = (1.0 / C) if c == 0 else 1.0
        for dw in range(7):
            jj = c * 7 + dw
            for dh in range(7):
                mconv[hh + dh, jj, hh] = sa_w[0, c, dh, dw] * cscale
    mconv = np.ascontiguousarray(mconv.reshape(70, 14 * 64)).astype(bf16)

    sabcol = np.full((64, 1), float(sa_b[0]), np.float32)
    w1col = np.full((64, 1), w1f, np.float32)
    w0col = np.full((128, 1), w0f, np.float32)

    return dict(
        w1sb=w1sb, b1col=b1col, gng=gng, gnb=gnb,
        sixt=np.full((CR, 1), 1.0 / CR, np.float32),
        zcol=np.zeros((128, 1), np.float32),
        w2b=w2b, mconv=mconv, sabcol=sabcol, w1col=w1col, w0col=w0col,
        onesr=np.ones((1, C), np.float32),
        cent=(np.eye(CR, dtype=np.float32) - 1.0 / CR),
    )


def _make_in_maps(inputs):
    """Shard + host-cast x to bf16 [C, BPC*HW] per core; bundle small weights."""
    import ml_dtypes

    bf16 = ml_dtypes.bfloat16
    x = np.asarray(inputs["x"], np.float32).reshape(B, C, HW)
    small = _host_prep(inputs)
    in_maps = []
    for i in range(NCORES):
        shard = x[i * BPC : (i + 1) * BPC]  # [BPC, C, HW]
        xs = np.ascontiguousarray(shard.transpose(1, 0, 2).reshape(C, FREE)).astype(bf16)
        m = dict(small)
        m["x"] = xs
        in_maps.append(m)
    return in_maps


def _gather_out(results):
    outs = []
    for i in range(NCORES):
        o = np.asarray(results[i]["out"], dtype=np.float32)  # [C, FREE] bf16 -> f32
        outs.append(o.reshape(C, BPC, HW).transpose(1, 0, 2))
    return np.concatenate(outs, axis=0).reshape(B, C, H, W)


def _run(inputs, trace=False):
    from concourse.bass_utils import run_bass_kernel_spmd

    nc = _build()
    in_maps = _make_in_maps(inputs)
    res = run_bass_kernel_spmd(nc, in_maps, core_ids=list(range(NCORES)), trace=trace)
    return _gather_out(res.results), res


def kernel(**inputs) -> np.ndarray:
    out, _ = _run(inputs, trace=False)
    return out


# revision 7
# speedup vs baseline: 2.3474x; 2.3474x over previous
"""Trainium2 Bass kernel for nn_AdaptiveAttention (dense_cnn, memory-bound).

out[b,c,h,w] = x[b,c,h,w] * (w0*ca[b,c] + w1*sa[b,h,w])

  ca = sigmoid(w2 @ silu(GN(w1 @ mean_hw(x) + b1)) + b2)      (channel attention)
  sa = sigmoid(conv7x7([mean_c(x), max_c(x)]) + sa_b)         (spatial attention)
  (w0, w1) = softmax(balance)

Data-parallel over batch: 8 NeuronCores x 4 batches each.

v4 design (CoreSim cost-model driven rebalance of v3):
  - channel max AND channel mean both via GPSIMD partition_all_reduce
    (1 elem/lane/cycle at 1.2 GHz): m=max(xt0,xt1), s=xt0+xt1 on DVE,
    then PAR(max)/PAR(add) on Pool; row 0 DMA'd into the zero-padded
    conv inputs. This deletes v3's 33 PE transposes, 24 DVE reduce_max,
    the avg-map PE matmuls and all 32 single-partition psum copies.
    The 1/C of the mean folds into the conv's avg-half weights.
  - apply: s_h = ACT(sab, Relu, bias=w0*ca_h) then one DVE bf16 mul per
    half; sab is the [1,4096] sa row broadcast via DRAM bounce re-read
    with a stride-0 partition AP (cheapest 128-way broadcast on TRN2).
  - DMA spread over 4 rings: x loads on sync, sab reads on scalar,
    stores split tensor/sync, pads/scr/consts on gpsimd.
  - squeeze (w1 @ mean_hw) stays on PE: 16 accumulating [16,512] psum
    matmuls per batch; GroupNorm(1)+SiLU chain 4-wide fp32 with a
    DVE-only Quake rsqrt; ca as per-half [128,1] columns.
"""

import sys

import numpy as np

if "/opt/trn_rl_repo" not in sys.path:
    sys.path.insert(0, "/opt/trn_rl_repo")

B, C, H, W = 32, 256, 64, 64
HW = H * W
CR = C // 16
NCORES = 8
BPC = B // NCORES
FREE = BPC * HW
CHUNK = 512
GN_EPS = 1e-5

import os
ABLATE = set(os.environ.get("KERNEL_ABLATE", "").split(",")) - {""}

_BUILT = None


def _build(loop_n=None):
    """Build (once) the SPMD Bass graph for one core's [256, 4*4096] bf16 shard."""
    global _BUILT
    if loop_n is None and _BUILT is not None:
        return _BUILT

    import concourse.bacc as bacc
    import concourse.tile as tile
    from concourse import mybir
    from concourse import bass_isa
    from concourse.bass import AP

    DT = mybir.dt.float32
    BT = mybir.dt.bfloat16
    AF = mybir.ActivationFunctionType
    AL = mybir.AluOpType
    AX = mybir.AxisListType

    nc = bacc.Bacc(
        "TRN2",
        target_bir_lowering=False,
        debug=False,
        enable_asserts=False,
        num_devices=NCORES,
    )

    x_d = nc.dram_tensor("x", [C, FREE], BT, kind="ExternalInput")
    w1sb_d = nc.dram_tensor("w1sb", [128, 2 * CR], BT, kind="ExternalInput")
    b1col_d = nc.dram_tensor("b1col", [CR, 1], DT, kind="ExternalInput")
    gng_d = nc.dram_tensor("gng", [CR, 1], DT, kind="ExternalInput")
    gnb_d = nc.dram_tensor("gnb", [CR, 1], DT, kind="ExternalInput")
    sixt_d = nc.dram_tensor("sixt", [CR, 1], DT, kind="ExternalInput")
    zcol_d = nc.dram_tensor("zcol", [128, 1], DT, kind="ExternalInput")
    w2b_d = nc.dram_tensor("w2b", [CR + 1, C], DT, kind="ExternalInput")
    mconv_d = nc.dram_tensor("mconv", [70, 14 * 64], BT, kind="ExternalInput")
    sabcol_d = nc.dram_tensor("sabcol", [64, 1], DT, kind="ExternalInput")
    w1col_d = nc.dram_tensor("w1col", [64, 1], DT, kind="ExternalInput")
    w0col_d = nc.dram_tensor("w0col", [128, 1], DT, kind="ExternalInput")
    onesr_d = nc.dram_tensor("onesr", [1, C], DT, kind="ExternalInput")
    cent_d = nc.dram_tensor("cent", [CR, CR], DT, kind="ExternalInput")
    scr_d = nc.dram_tensor("scr", [BPC, HW], BT, kind="Internal")
    out_d = nc.dram_tensor("out", [C, FREE], BT, kind="ExternalOutput")

    with tile.TileContext(nc) as tc:
        with (
            tc.tile_pool(name="const", bufs=1) as constp,
            tc.tile_pool(name="xp", bufs=8) as xp,
            tc.tile_pool(name="outp", bufs=3) as outp,
            tc.tile_pool(name="prep", bufs=3) as prep,
            tc.tile_pool(name="parp", bufs=2) as parp,
            tc.tile_pool(name="sabp", bufs=2) as sabp,
            tc.tile_pool(name="sp", bufs=4) as sp,
            tc.tile_pool(name="phps", bufs=2, space="PSUM") as phps,
            tc.tile_pool(name="psml", bufs=2, space="PSUM") as psml,
            tc.tile_pool(name="pconv", bufs=2, space="PSUM") as pconv,
        ):
            # ---- constants (DMA'd once, off the hot rings) ----
            w1sb_t = constp.tile([128, 2 * CR], BT, name="w1sb_t")
            nc.gpsimd.dma_start(out=w1sb_t[:], in_=w1sb_d[:])
            b1col_t = constp.tile([CR, 1], DT, name="b1col_t")
            nc.gpsimd.dma_start(out=b1col_t[:], in_=b1col_d[:])
            gng_t = constp.tile([CR, 1], DT, name="gng_t")
            nc.gpsimd.dma_start(out=gng_t[:], in_=gng_d[:])
            gnb_t = constp.tile([CR, 1], DT, name="gnb_t")
            nc.gpsimd.dma_start(out=gnb_t[:], in_=gnb_d[:])
            sixt_t = constp.tile([CR, 1], DT, name="sixt_t")
            nc.gpsimd.dma_start(out=sixt_t[:], in_=sixt_d[:])
            zcol_t = constp.tile([128, 1], DT, name="zcol_t")
            nc.gpsimd.dma_start(out=zcol_t[:], in_=zcol_d[:])
            w2b_t = constp.tile([CR + 1, C], DT, name="w2b_t")
            nc.gpsimd.dma_start(out=w2b_t[:], in_=w2b_d[:])
            mconv_t = constp.tile([70, 14 * 64], BT, name="mconv_t")
            nc.gpsimd.dma_start(out=mconv_t[:], in_=mconv_d[:])
            sabcol_t = constp.tile([64, 1], DT, name="sabcol_t")
            nc.gpsimd.dma_start(out=sabcol_t[:], in_=sabcol_d[:])
            w1col_t = constp.tile([64, 1], DT, name="w1col_t")
            nc.gpsimd.dma_start(out=w1col_t[:], in_=w1col_d[:])
            w0col_t = constp.tile([128, 1], DT, name="w0col_t")
            nc.gpsimd.dma_start(out=w0col_t[:], in_=w0col_d[:])
            onesr_t = constp.tile([1, C], DT, name="onesr_t")
            nc.gpsimd.dma_start(out=onesr_t[:], in_=onesr_d[:])
            cent16_t = constp.tile([CR, CR], DT, name="cent16_t")
            nc.gpsimd.dma_start(out=cent16_t[:], in_=cent_d[:])

            # persistent zero-bordered conv inputs (interior rewritten per batch)
            pads_av, pads_mx = [], []
            for b in range(BPC):
                pad_av = constp.tile([70, 70], BT, name=f"padav_{b}")
                nc.gpsimd.memset(pad_av[:, :], 0.0)
                pad_mx = constp.tile([70, 70], BT, name=f"padmx_{b}")
                nc.gpsimd.memset(pad_mx[:, :], 0.0)
                pads_av.append(pad_av); pads_mx.append(pad_mx)
            hb4 = constp.tile([CR + 1, BPC], DT, name="hb4")
            nc.gpsimd.dma_start(out=hb4[CR : CR + 1, :], in_=onesr_d[0:1, 0:BPC])

            import contextlib
            loop_cm = tc.For_i(0, loop_n, 1) if loop_n is not None else contextlib.nullcontext()
            with loop_cm:
              hred4 = sp.tile([CR, BPC], DT, tag="hred4", name="hred4", bufs=2)
              xts, sabs = [], []
              for b in range(BPC):
                pad_av, pad_mx = pads_av[b], pads_mx[b]
                bsl = slice(b * HW, (b + 1) * HW)
                xt0 = xp.tile([128, HW], BT, tag="xt", name=f"xt0_{b}")
                xt1 = xp.tile([128, HW], BT, tag="xt", name=f"xt1_{b}")
                nc.sync.dma_start(out=xt0[:, :], in_=x_d[0:128, bsl])
                nc.sync.dma_start(out=xt1[:, :], in_=x_d[128:256, bsl])

                # channel max / channel sum maps via GPSIMD partition reduce
                m = prep.tile([128, HW], BT, tag="pre", name=f"m_{b}")
                nc.vector.tensor_max(m[:, :], xt0[:, :], xt1[:, :])
                s = prep.tile([128, HW], BT, tag="pre", name=f"s_{b}")
                nc.vector.tensor_add(s[:, :], xt0[:, :], xt1[:, :])
                if "par" not in ABLATE:
                    mx = parp.tile([128, HW], BT, tag="par", name=f"mx_{b}")
                    nc.gpsimd.partition_all_reduce(
                        mx[:, :], m[:, :], channels=128, reduce_op=bass_isa.ReduceOp.max
                    )
                    nc.gpsimd.dma_start(out=pad_mx[3:67, 3:67], in_=mx[0:1, :])
                    sm = parp.tile([128, HW], BT, tag="par", name=f"sm_{b}")
                    nc.gpsimd.partition_all_reduce(
                        sm[:, :], s[:, :], channels=128, reduce_op=bass_isa.ReduceOp.add
                    )
                    nc.gpsimd.dma_start(out=pad_av[3:67, 3:67], in_=sm[0:1, :])

                # squeeze: h_partial[CR, 512] accumulated over halves+chunks
                hps = phps.tile([CR, CHUNK], DT, tag="hps", name=f"hps_{b}")
                nmm = 0
                for j in range(8):
                    sl = slice(j * CHUNK, (j + 1) * CHUNK)
                    nc.tensor.matmul(
                        hps[:, :], w1sb_t[:, 0:CR], xt0[:, sl],
                        start=(nmm == 0), stop=False,
                    )
                    nmm += 1
                    nc.tensor.matmul(
                        hps[:, :], w1sb_t[:, CR : 2 * CR], xt1[:, sl],
                        start=False, stop=(nmm == 15),
                    )
                    nmm += 1
                nc.vector.reduce_sum(hred4[:, b : b + 1], hps[:, :], axis=AX.X)

                # ---- spatial attention: 7x7 conv as 14 banded bf16 matmuls ----
                convps = pconv.tile([64, 64], DT, tag="conv", name=f"convps_{b}")
                idx = 0
                for ci, pad in ((0, pad_av), (1, pad_mx)):
                    for dw in range(7):
                        jj = ci * 7 + dw
                        nc.tensor.matmul(
                            convps[:, :],
                            mconv_t[:, 64 * jj : 64 * (jj + 1)],
                            pad[:, dw : dw + 64],
                            start=(idx == 0),
                            stop=(idx == 13),
                        )
                        idx += 1
                sasig = sp.tile([64, 64], BT, tag="sasig", name=f"sasig_{b}")
                nc.scalar.activation(sasig[:], convps[:], AF.Sigmoid, bias=sabcol_t[:])
                sasw = sp.tile([64, 64], BT, tag="sasw", name=f"sasw_{b}")
                nc.vector.tensor_scalar_mul(sasw[:], sasig[:], w1col_t[:, 0:1])

                # broadcast w1*sa to all 128 partitions via DRAM bounce
                nc.gpsimd.dma_start(out=scr_d[b : b + 1, :], in_=sasw[:, :])
                sab = sabp.tile([128, HW], BT, tag="sab", name=f"sab_{b}")
                src = scr_d[b : b + 1, :]
                bsrc = AP(src.tensor, src.offset, [[0, 128]] + list(src.ap)[1:])
                nc.scalar.dma_start(out=sab[:, :], in_=bsrc)
                xts.append((xt0, xt1)); sabs.append(sab)

              # ---- 4-wide GroupNorm(1)+SiLU chain (all batches at once) ----
              hcol4 = sp.tile([CR, BPC], DT, tag="hcol4", name="hcol4")
              nc.vector.tensor_scalar_add(hcol4[:, :], hred4[:, :], b1col_t[:, 0:1])
              dps4 = psml.tile([CR, BPC], DT, tag="sps", name="dps4")
              nc.tensor.matmul(dps4[:, :], cent16_t[:, :], hcol4[:, :])
              diff4 = sp.tile([CR, BPC], DT, tag="diff4", name="diff4")
              nc.scalar.copy(diff4[:], dps4[:])
              sq4 = sp.tile([CR, BPC], DT, tag="sq4", name="sq4")
              nc.vector.tensor_mul(sq4[:], diff4[:], diff4[:])
              vrow = psml.tile([1, BPC], DT, tag="sps", name="vrow")
              nc.tensor.matmul(vrow[:, :], sixt_t[:, :], sq4[:, :])
              vsb4 = sp.tile([1, BPC], DT, tag="vsb4", name="vsb4")
              nc.scalar.copy(vsb4[:], vrow[:])
              vm4 = sp.tile([1, BPC], DT, tag="vm4", name="vm4")
              nc.vector.tensor_scalar(vm4[:], vsb4[:], 1.0, GN_EPS, op0=AL.mult, op1=AL.add)
              vh4 = sp.tile([1, BPC], DT, tag="vh4", name="vh4")
              nc.vector.tensor_scalar_mul(vh4[:], vm4[:], 0.5)
              # Quake rsqrt on the [1,4] row; two Newton steps (sign restores)
              rs4 = sp.tile([1, BPC], DT, tag="rs4", name="rs4")
              rs4_i = rs4.bitcast(mybir.dt.int32)
              vm4_i = vm4.bitcast(mybir.dt.int32)
              nc.vector.tensor_scalar(rs4_i[:], vm4_i[:], 1, None, op0=AL.arith_shift_right)
              nc.vector.tensor_scalar(rs4_i[:], rs4_i[:], 0x5F3759DF, -1, op0=AL.subtract, op1=AL.mult)
              for it in range(2):
                  ysq4 = sp.tile([1, BPC], DT, tag="ysq4", name=f"ysq4_{it}")
                  nc.vector.tensor_mul(ysq4[:], rs4[:], rs4[:])
                  u4 = sp.tile([1, BPC], DT, tag="u4", name=f"u4_{it}")
                  nc.vector.tensor_mul(u4[:], ysq4[:], vh4[:])
                  nc.vector.tensor_scalar(u4[:], u4[:], 1.5, None, op0=AL.subtract)
                  nc.vector.tensor_mul(rs4[:], rs4[:], u4[:])
              rs16 = psml.tile([CR, BPC], DT, tag="sps", name="rs16")
              nc.tensor.matmul(rs16[:, :], onesr_t[0:1, 0:CR], rs4[:, :])
              rsc4 = sp.tile([CR, BPC], DT, tag="rsc4", name="rsc4")
              nc.scalar.copy(rsc4[:], rs16[:])
              hn4 = sp.tile([CR, BPC], DT, tag="hn4", name="hn4")
              nc.vector.tensor_mul(hn4[:], diff4[:], rsc4[:])
              hg24 = sp.tile([CR, BPC], DT, tag="hg24", name="hg24")
              nc.vector.tensor_scalar(hg24[:], hn4[:], gng_t[:, 0:1], gnb_t[:, 0:1], op0=AL.mult, op1=AL.add)
              sg4 = sp.tile([CR, BPC], DT, tag="sg4", name="sg4")
              nc.scalar.activation(sg4[:], hg24[:], AF.Sigmoid, bias=zcol_t[0:CR, 0:1])
              nc.vector.tensor_mul(hb4[0:CR, :], hg24[:], sg4[:])

              # ca columns for all batches: [128, 4] per half, kept fp32
              caw4 = []
              for h in range(2):
                  capsh = psml.tile([128, BPC], DT, tag="sps", name=f"caps4_{h}")
                  nc.tensor.matmul(capsh[:, :], w2b_t[:, 128 * h : 128 * (h + 1)], hb4[:, :])
                  casgh = sp.tile([128, BPC], DT, tag=f"casg{h}", name=f"casg4_{h}")
                  nc.scalar.activation(casgh[:], capsh[:], AF.Sigmoid, bias=zcol_t[:, 0:1])
                  cwh = sp.tile([128, BPC], DT, tag=f"caw{h}", name=f"caw4_{h}")
                  nc.vector.tensor_scalar_mul(cwh[:], casgh[:], w0col_t[:, 0:1])
                  caw4.append(cwh)

              # ---- apply: S = relu(sab + w0*ca) on ACT, one bf16 DVE mul per
              # half; stores split across the tensor and sync rings.
              for b in range(BPC):
                  bsl = slice(b * HW, (b + 1) * HW)
                  xt0, xt1 = xts[b]
                  sab = sabs[b]
                  s0 = outp.tile([128, HW], BT, tag="s0", name=f"s0_{b}", bufs=2)
                  nc.scalar.activation(s0[:, :], sab[:, :], AF.Relu, bias=caw4[0][:, b : b + 1])
                  s1 = outp.tile([128, HW], BT, tag="s1", name=f"s1_{b}", bufs=2)
                  nc.vector.tensor_scalar_add(s1[:, :], sab[:, :], caw4[1][:, b : b + 1])
                  ot0 = outp.tile([128, HW], BT, tag="ot", name=f"ot0_{b}")
                  ot1 = outp.tile([128, HW], BT, tag="ot", name=f"ot1_{b}")
                  nc.vector.tensor_mul(ot0[:, :], s0[:, :], xt0[:, :])
                  nc.vector.tensor_mul(ot1[:, :], s1[:, :], xt1[:, :])
                  nc.scalar.dma_start(out=out_d[0:128, bsl], in_=ot0[:, :])
                  nc.sync.dma_start(out=out_d[128:256, bsl], in_=ot1[:, :])

    nc.compile()
    if loop_n is None:
        _BUILT = nc
    return nc


def _host_prep(inputs):
    """Host-side prep of the tiny weight tensors into matmul-ready layouts."""
    import ml_dtypes

    bf16 = ml_dtypes.bfloat16

    w1 = np.asarray(inputs["w1"], np.float32)
    b1 = np.asarray(inputs["b1"], np.float32)
    gn_g = np.asarray(inputs["gn_g"], np.float32)
    gn_b = np.asarray(inputs["gn_b"], np.float32)
    w2 = np.asarray(inputs["w2"], np.float32)
    b2 = np.asarray(inputs["b2"], np.float32)
    sa_w = np.asarray(inputs["sa_w"], np.float32)
    sa_b = np.asarray(inputs["sa_b"], np.float32)
    balance = np.asarray(inputs["balance"], np.float64)

    e = np.exp(balance - balance.max())
    wsm = e / e.sum()
    w0f, w1f = float(wsm[0]), float(wsm[1])

    w1sb = np.zeros((128, 2 * CR), np.float32)
    for ct in range(2):
        w1sb[:, CR * ct : CR * (ct + 1)] = w1[:, 128 * ct : 128 * (ct + 1)].T / HW
    w1sb = w1sb.astype(bf16)
    b1col = b1.reshape(CR, 1).copy()
    gng = gn_g.reshape(CR, 1).copy()
    gnb = gn_b.reshape(CR, 1).copy()
    w2b = np.concatenate([w2.T, b2.reshape(1, C)], axis=0).astype(np.float32)

    # banded H-conv matrices: M_{c,dw}[k, h] = sa_w[0, c, k-h, dw], 0<=k-h<7
    # the avg-channel (c=0) absorbs the 1/C of mean_c (conv input is the SUM map)
    mconv = np.zeros((70, 14, 64), np.float32)
    hh = np.arange(64)
    for c in range(2):
        cscale = (1.0 / C) if c == 0 else 1.0
        for dw in range(7):
            jj = c * 7 + dw
            for dh in range(7):
                mconv[hh + dh, jj, hh] = sa_w[0, c, dh, dw] * cscale
    mconv = np.ascontiguousarray(mconv.reshape(70, 14 * 64)).astype(bf16)

    sabcol = np.full((64, 1), float(sa_b[0]), np.float32)
    w1col = np.full((64, 1), w1f, np.float32)
    w0col = np.full((128, 1), w0f, np.float32)

    return dict(
        w1sb=w1sb, b1col=b1col, gng=gng, gnb=gnb,
        sixt=np.full((CR, 1), 1.0 / CR, np.float32),
        zcol=np.zeros((128, 1), np.float32),
        w2b=w2b, mconv=mconv, sabcol=sabcol, w1col=w1col, w0col=w0col,
        onesr=np.ones((1, C), np.float32),
        cent=(np.eye(CR, dtype=np.float32) - 1.0 / CR),
    )


def _make_in_maps(inputs):
    """Shard + host-cast x to bf16 [C, BPC*HW] per core; bundle small weights."""
    import ml_dtypes

    bf16 = ml_dtypes.bfloat16
    x = np.asarray(inputs["x"], np.float32).reshape(B, C, HW)
    small = _host_prep(inputs)
    in_maps = []
    for i in range(NCORES):
        shard = x[i * BPC : (i + 1) * BPC]  # [BPC, C, HW]
        xs = np.ascontiguousarray(shard.transpose(1, 0, 2).reshape(C, FREE)).astype(bf16)
        m = dict(small)
        m["x"] = xs
        in_maps.append(m)
    return in_maps


def _gather_out(results):
    outs = []
    for i in range(NCORES):
        o = np.asarray(results[i]["out"], dtype=np.float32)  # [C, FREE] bf16 -> f32
        outs.append(o.reshape(C, BPC, HW).transpose(1, 0, 2))
    return np.concatenate(outs, axis=0).reshape(B, C, H, W)


def _run(inputs, trace=False):
    from concourse.bass_utils import run_bass_kernel_spmd

    nc = _build()
    in_maps = _make_in_maps(inputs)
    res = run_bass_kernel_spmd(nc, in_maps, core_ids=list(range(NCORES)), trace=trace)
    return _gather_out(res.results), res


def kernel(**inputs) -> np.ndarray:
    out, _ = _run(inputs, trace=False)
    return out
